# revision 11
# baseline (speedup 1.0000x reference)
"""Trainium2 Bass kernel for CRF mean log-likelihood (B=128, S=512, T=256).

Strategy: data-parallel over batch (16 sequences per core, 8 cores). The
forward-algorithm log-partition is computed in exponential space so the
per-step T x T logsumexp becomes a PE matmul:

    alpha_{s+1} = log( E^T @ exp(alpha_s) ) + emit_{s+1}        E = exp(trans)
 => p_{s+1} = (E^T @ p_s) * exp(emit_{s+1} - delta)             p = exp-space state

with a constant per-step shift delta ~= log(T) + 1/2 (keeps p in a narrow
dynamic range; validated drift < +-6 in log space) and an exact
renormalization every R steps for safety (sum via ones-matmul, folded into
the next step's emission factor, so it costs nothing on the critical path).

Per core/step: 4 accumulating matmuls (E tiles bf16 stationary [128,128],
p bf16 moving [128,8] per batch-group) + 1 DVE multiply per group
(PSUM f32 q times SBUF bf16 ee -> bf16 p'). Emissions are streamed in
f32 chunks and exponentiated on the ScalarEngine ahead of use.

The gold (numerator) score is O(B*S) gather work — computed on host.
"""
import os
import numpy as np

B, S, T = 128, 512, 256
NCORES = 8
BPC = B // NCORES          # batch per core = 16
G = 2                      # interleaved batch groups per core (latency hiding)
GB = BPC // G              # batch per group = 8
W = 64                     # steps per emissions chunk
R = 64                     # renormalization period
DELTA = 6.045              # per-step log-space shift ~ log(256) + 0.5

_cache = {}


def build_nc(n_steps=S):
    import concourse.bass as bass
    import concourse.tile as tile
    from concourse import bacc, mybir
    from contextlib import ExitStack

    f32 = mybir.dt.float32
    bf16 = mybir.dt.bfloat16
    Exp = mybir.ActivationFunctionType.Exp
    Ln = mybir.ActivationFunctionType.Ln

    nc = bacc.Bacc()
    em = nc.declare_dram_parameter("em", [2, 128, n_steps, BPC], f32, isOutput=False)
    tr = nc.declare_dram_parameter("tr", [2, 128, 2, 128], f32, isOutput=False)
    stw = nc.declare_dram_parameter("stw", [2, 128, 1], f32, isOutput=False)
    enw = nc.declare_dram_parameter("enw", [2, 128, 1], f32, isOutput=False)
    out = nc.declare_dram_parameter("out", [1, BPC], f32, isOutput=True)

    n_chunks = (n_steps + W - 1) // W

    with ExitStack() as ctx:
        tc = ctx.enter_context(tile.TileContext(nc))
        const = ctx.enter_context(tc.tile_pool(name="const", bufs=1))
        empool = ctx.enter_context(tc.tile_pool(name="em", bufs=2))
        eepool = ctx.enter_context(tc.tile_pool(name="ee", bufs=2))
        ppool = ctx.enter_context(tc.tile_pool(name="p", bufs=3))
        rpool = ctx.enter_context(tc.tile_pool(name="rn", bufs=2))
        qpool = ctx.enter_context(tc.tile_pool(name="q", bufs=2, space="PSUM"))
        spool = ctx.enter_context(tc.tile_pool(name="s", bufs=2, space="PSUM"))

        # ---- one-time constants ----
        E = [[None, None], [None, None]]  # E[i][j]: k-chunk i, m-chunk j
        for i in range(2):
            stage = rpool.tile([128, 2, 128], f32, tag="trstage", name="trstage")
            nc.sync.dma_start(out=stage, in_=tr[i])
            for j in range(2):
                E[i][j] = const.tile([128, 128], bf16, tag=f"E{i}{j}", name=f"E{i}{j}")
                nc.scalar.activation(E[i][j], stage[:, j, :], Exp)
        ones = const.tile([128, 128], bf16, tag="ones", name="ones")
        nc.vector.memset(ones, 1.0)
        st_t = []
        for i in range(2):
            t = const.tile([128, 1], f32, tag=f"st{i}", name=f"st{i}")
            nc.sync.dma_start(out=t, in_=stw[i])
            st_t.append(t)
        en_t = []
        for i in range(2):
            stage = rpool.tile([128, 1], f32, tag="enstage", name="enstage")
            nc.sync.dma_start(out=stage, in_=enw[i])
            t = const.tile([128, 1], bf16, tag=f"en{i}", name=f"en{i}")
            nc.scalar.activation(t, stage, Exp)
            en_t.append(t)
        acc = []
        for g in range(G):
            a = const.tile([1, GB], f32, tag=f"acc{g}", name=f"acc{g}")
            nc.vector.memset(a, 0.0)
            acc.append(a)
        dbias = const.tile([128, 1], f32, tag="dbias", name="dbias")
        nc.vector.memset(dbias, -DELTA)

        # ---- emissions chunk streaming ----
        def load_chunk(c):
            s0, s1 = c * W, min(n_steps, (c + 1) * W)
            t = empool.tile([128, 2, W, BPC], f32, tag="emchunk", name="emchunk")
            for i in range(2):
                nc.sync.dma_start(out=t[:, i, 0:s1 - s0, :], in_=em[i, :, s0:s1, :])
            return t

        def exp_chunk(c, em_t):
            s0, s1 = c * W, min(n_steps, (c + 1) * W)
            t = eepool.tile([128, 2, W, BPC], bf16, tag="eechunk", name="eechunk")
            nc.scalar.activation(t[:, :, 0:s1 - s0, :], em_t[:, :, 0:s1 - s0, :],
                                 Exp, bias=dbias)
            return t

        em_t0 = load_chunk(0)
        # init p from step 0: p = exp(emissions[:,0,:] + start)
        p = []
        for g in range(G):
            pt = ppool.tile([128, 2, GB], bf16, tag=f"p{g}", name=f"p{g}")
            for i in range(2):
                nc.scalar.activation(pt[:, i, :],
                                     em_t0[:, i, 0, g * GB:(g + 1) * GB],
                                     Exp, bias=st_t[i])
            p.append(pt)
        ee_cur = exp_chunk(0, em_t0)

        rec_pending = None
        for s in range(1, n_steps):
            c, w = divmod(s, W)
            if w == 0:
                em_tc = load_chunk(c)
                ee_cur = exp_chunk(c, em_tc)
            qs = [qpool.tile([128, 2, GB], f32, tag=f"q{g}", name=f"q{g}") for g in range(G)]
            # weight-grouped matmul order: each E tile loaded once per round
            for j in range(2):
                for i in range(2):
                    for g in range(G):
                        nc.tensor.matmul(qs[g][:, j, :], E[i][j], p[g][:, i, :],
                                         start=(i == 0), stop=(i == 1))
            newp = []
            for g in range(G):
                eesl = ee_cur[:, :, w, g * GB:(g + 1) * GB]
                if rec_pending is not None:
                    ee2 = rpool.tile([128, 2, GB], bf16, tag=f"eesc{g}", name=f"eesc{g}")
                    for i in range(2):
                        nc.vector.tensor_mul(ee2[:, i, :], eesl[:, i, :],
                                             rec_pending[g])
                    eesl = ee2
                pt = ppool.tile([128, 2, GB], bf16, tag=f"p{g}", name=f"p{g}")
                nc.vector.tensor_mul(pt, qs[g], eesl)
                newp.append(pt)
            p = newp
            rec_pending = None
            if s % R == 0 and s + 1 < n_steps:
                rec_pending = []
                for g in range(G):
                    sp = spool.tile([128, GB], f32, tag="sump", name="sump")
                    for i in range(2):
                        nc.tensor.matmul(sp, ones, p[g][:, i, :],
                                         start=(i == 0), stop=(i == 1))
                    rc = rpool.tile([128, GB], f32, tag=f"rec{g}", name=f"rec{g}")
                    nc.vector.reciprocal(rc, sp)
                    rec_pending.append(rc)
                    lg = rpool.tile([1, GB], f32, tag=f"lg{g}", name=f"lg{g}")
                    nc.scalar.activation(lg, sp[0:1, :], Ln)
                    nc.vector.tensor_add(acc[g], acc[g], lg)

        # ---- final: log(sum_t p * exp(end_t)) + acc ----
        for g in range(G):
            fp = spool.tile([1, GB], f32, tag="sump", name="finp")
            for i in range(2):
                nc.tensor.matmul(fp, en_t[i], p[g][:, i, :],
                                 start=(i == 0), stop=(i == 1))
            fl = rpool.tile([1, GB], f32, tag=f"fl{g}", name=f"fl{g}")
            nc.scalar.activation(fl, fp, Ln)
            res = rpool.tile([1, GB], f32, tag=f"res{g}", name=f"res{g}")
            nc.vector.tensor_add(res, fl, acc[g])
            nc.sync.dma_start(out=out[0:1, g * GB:(g + 1) * GB], in_=res)
    nc.compile()
    return nc


def _prep_inputs(emissions, transitions, start_transitions, end_transitions,
                 n_steps=S):
    """Host-side layout prep: per-core input maps."""
    emissions = np.ascontiguousarray(emissions[:, :n_steps, :], dtype=np.float32)
    em_t = np.ascontiguousarray(emissions.transpose(2, 1, 0)).reshape(
        2, 128, n_steps, B)  # [i, p, s, b]
    tr = np.ascontiguousarray(
        np.asarray(transitions, np.float32).reshape(2, 128, 2, 128))
    stw = np.ascontiguousarray(
        np.asarray(start_transitions, np.float32).reshape(2, 128, 1))
    enw = np.ascontiguousarray(
        np.asarray(end_transitions, np.float32).reshape(2, 128, 1))
    in_maps = []
    for c in range(NCORES):
        in_maps.append({
            "em": np.ascontiguousarray(em_t[:, :, :, c * BPC:(c + 1) * BPC]),
            "tr": tr, "stw": stw, "enw": enw,
        })
    return in_maps


def _gold_score_host(emissions, tags, mask, transitions, start_transitions,
                     end_transitions):
    emissions = np.asarray(emissions, np.float32)
    tags = np.asarray(tags, np.int64)
    m = np.asarray(mask, np.float32)
    emit = np.take_along_axis(emissions, tags[..., None], axis=2)[..., 0]
    trans = np.asarray(transitions, np.float32)[tags[:, :-1], tags[:, 1:]]
    score = (np.asarray(start_transitions, np.float32)[tags[:, 0]] + emit[:, 0]
             + ((emit[:, 1:] + trans) * m[:, 1:]).sum(axis=1))
    last_idx = np.asarray(mask, np.int64).sum(axis=1) - 1
    last_tags = np.take_along_axis(tags, last_idx[:, None], axis=1)[:, 0]
    return score + np.asarray(end_transitions, np.float32)[last_tags]


def _numpy_fallback(emissions, tags, mask, transitions, start_transitions,
                    end_transitions):
    """Reference-faithful numpy path (only used if mask is not all ones)."""
    em = np.asarray(emissions, np.float64)
    msk = np.asarray(mask, bool)
    trn = np.asarray(transitions, np.float64)
    alpha = np.asarray(start_transitions, np.float64)[None, :] + em[:, 0]
    for s in range(1, em.shape[1]):
        scores = alpha[:, :, None] + trn[None, :, :] + em[:, s][:, None, :]
        mx = scores.max(axis=1, keepdims=True)
        new = np.log(np.exp(scores - mx).sum(axis=1)) + mx[:, 0, :]
        alpha = np.where(msk[:, s][:, None], new, alpha)
    fin = alpha + np.asarray(end_transitions, np.float64)[None, :]
    mx = fin.max(axis=1, keepdims=True)
    logden = np.log(np.exp(fin - mx).sum(axis=1)) + mx[:, 0]
    gold = _gold_score_host(emissions, tags, mask, transitions,
                            start_transitions, end_transitions)
    return np.array(np.mean(gold - logden), dtype=np.float32)


def run_device(emissions, transitions, start_transitions, end_transitions,
               n_steps=S, trace=False, tmpdir=None):
    """Compile (cached) + run the Bass kernel; returns (logden[B], results_obj)."""
    from concourse.bass_utils import run_bass_kernel_spmd
    key = n_steps
    if key not in _cache:
        _cache[key] = build_nc(n_steps)
    nc = _cache[key]
    in_maps = _prep_inputs(emissions, transitions, start_transitions,
                           end_transitions, n_steps)
    core_ids = list(range(NCORES))
    r = run_bass_kernel_spmd(nc, in_maps, core_ids, trace=trace, tmpdir=tmpdir)
    logden = np.concatenate([np.asarray(r.results[c]["out"][0], np.float32)
                             for c in range(NCORES)])
    logden = logden + np.float32((n_steps - 1) * DELTA)
    return logden, r


def kernel(emissions, tags, mask, transitions, start_transitions,
           end_transitions):
    emissions = np.asarray(emissions)
    tags = np.asarray(tags)
    mask = np.asarray(mask)
    if not mask.all():
        return _numpy_fallback(emissions, tags, mask, transitions,
                               start_transitions, end_transitions)
    logden, _ = run_device(emissions, transitions, start_transitions,
                           end_transitions)
    gold = _gold_score_host(emissions, tags, mask, transitions,
                            start_transitions, end_transitions)
    return np.array(np.mean(gold - logden), dtype=np.float32)


# revision 12
# speedup vs baseline: 1.8555x; 1.8555x over previous
"""Trainium2 Bass kernel for CRF mean log-likelihood (B=128, S=512, T=256).

Strategy: data-parallel over batch (16 sequences per core, 8 cores). The
forward-algorithm log-partition is computed in exponential space so the
per-step T x T logsumexp becomes a PE matmul:

    alpha_s = (E^T alpha_{s-1}) * exp(emit_s - delta)     E = exp(trans)

with a constant per-step shift delta ~= log(T) + 1/2 (keeps the state in a
narrow dynamic range; validated drift < +-6 in log space) and an exact
renormalization every R steps for safety (sum via ones-matmul, folded into
the next step's emission factor, off the critical path).

The chain is latency-bound (matmul -> DVE multiply -> matmul), so the
sequence is processed FROM BOTH ENDS simultaneously (meet in the middle):
  forward:  alpha_s = (E^T alpha_{s-1}) . ee_s          s = 1..Rf
  backward: u_s = (E u_{s+1}) . ee_s   (u_s=gamma_s.ee_s), s = S-2..Rf+1
  Z        = (E^T alpha_Rf)^T  u_{Rf+1}
Two independent chains per batch group halve the sequential depth.

The gold (numerator) score is O(B*S) gather work — computed on host.
"""
import numpy as np

B, S, T = 128, 512, 256
NCORES = 8
BPC = B // NCORES          # batch per core = 16
G = 1                      # batch groups per core (chains = 2*G)
GB = BPC // G
W = 64                     # steps per emissions chunk
R = 64                     # renormalization period
DELTA = 6.045              # per-step log-space shift ~ log(256) + 0.5
KEEP_MM_WAITS = True       # skip bacc's move_matmul_waits_to_ldweights

_cache = {}


def build_nc(n_steps=S):
    import concourse.bass as bass
    import concourse.tile as tile
    from concourse import bacc, mybir
    from contextlib import ExitStack

    f32 = mybir.dt.float32
    bf16 = mybir.dt.bfloat16
    Exp = mybir.ActivationFunctionType.Exp
    Ln = mybir.ActivationFunctionType.Ln

    assert n_steps >= 4
    Rf = (n_steps - 2) // 2          # forward DVE-rounds (alpha_1..alpha_Rf)
    Rb = n_steps - 2 - Rf            # backward rounds (u_{S-2}..u_{Rf+1})

    nc = bacc.Bacc()
    em = nc.declare_dram_parameter("em", [2, 128, n_steps, BPC], f32, isOutput=False)
    tr = nc.declare_dram_parameter("tr", [2, 128, 2, 128], f32, isOutput=False)
    trt = nc.declare_dram_parameter("trt", [2, 128, 2, 128], f32, isOutput=False)
    stw = nc.declare_dram_parameter("stw", [2, 128, 1], f32, isOutput=False)
    enw = nc.declare_dram_parameter("enw", [2, 128, 1], f32, isOutput=False)
    out = nc.declare_dram_parameter("out", [1, BPC], f32, isOutput=True)

    with ExitStack() as ctx:
        tc = ctx.enter_context(tile.TileContext(nc))
        const = ctx.enter_context(tc.tile_pool(name="const", bufs=1))
        emf = ctx.enter_context(tc.tile_pool(name="emf", bufs=2))
        eef = ctx.enter_context(tc.tile_pool(name="eef", bufs=2))
        emb = ctx.enter_context(tc.tile_pool(name="emb", bufs=2))
        eeb = ctx.enter_context(tc.tile_pool(name="eeb", bufs=2))
        ppool = ctx.enter_context(tc.tile_pool(name="p", bufs=3))
        rpool = ctx.enter_context(tc.tile_pool(name="rn", bufs=2))
        qpool = ctx.enter_context(tc.tile_pool(name="q", bufs=1, space="PSUM"))
        spool = ctx.enter_context(tc.tile_pool(name="s", bufs=2, space="PSUM"))

        # ---- one-time constants ----
        E = [[None, None], [None, None]]   # E[i][j]: lhsT for forward
        Et = [[None, None], [None, None]]  # Et[i][j]: lhsT for backward
        for i in range(2):
            stage = rpool.tile([128, 2, 128], f32, tag="trstage", name="trstage")
            nc.sync.dma_start(out=stage, in_=tr[i])
            for j in range(2):
                E[i][j] = const.tile([128, 128], bf16, tag=f"E{i}{j}", name=f"E{i}{j}")
                nc.scalar.activation(E[i][j], stage[:, j, :], Exp)
        for i in range(2):
            stage = rpool.tile([128, 2, 128], f32, tag="trstage", name="trstaget")
            nc.sync.dma_start(out=stage, in_=trt[i])
            for j in range(2):
                Et[i][j] = const.tile([128, 128], bf16, tag=f"Et{i}{j}",
                                      name=f"Et{i}{j}")
                nc.scalar.activation(Et[i][j], stage[:, j, :], Exp)
        ones = const.tile([128, 128], bf16, tag="ones", name="ones")
        nc.vector.memset(ones, 1.0)
        onesf = const.tile([128, 1], f32, tag="onesf", name="onesf")
        nc.vector.memset(onesf, 1.0)
        dbias = const.tile([128, 1], f32, tag="dbias", name="dbias")
        nc.vector.memset(dbias, -DELTA)
        st_t = []
        for i in range(2):
            t = const.tile([128, 1], f32, tag=f"st{i}", name=f"st{i}")
            nc.sync.dma_start(out=t, in_=stw[i])
            st_t.append(t)
        ben = []
        for i in range(2):
            stage = rpool.tile([128, 1], f32, tag="enstage", name="enstage")
            nc.sync.dma_start(out=stage, in_=enw[i])
            t = const.tile([128, 1], f32, tag=f"ben{i}", name=f"ben{i}")
            nc.vector.tensor_add(t, stage, dbias)   # end - delta (bwd init bias)
            ben.append(t)
        accs = {}
        for d in ("f", "b"):
            for g in range(G):
                a = const.tile([1, GB], f32, tag=f"acc{d}{g}", name=f"acc{d}{g}")
                nc.vector.memset(a, 0.0)
                accs[(d, g)] = a

        # ---- emissions chunk streaming (per direction) ----
        def load_chunk(c, pool, nm):
            s0, s1 = c * W, min(n_steps, (c + 1) * W)
            t = pool.tile([128, 2, W, BPC], f32, tag="emchunk", name=f"em{nm}")
            for i in range(2):
                nc.sync.dma_start(out=t[:, i, 0:s1 - s0, :], in_=em[i, :, s0:s1, :])
            return t

        def exp_chunk(c, em_t, pool, nm):
            s0, s1 = c * W, min(n_steps, (c + 1) * W)
            t = pool.tile([128, 2, W, BPC], bf16, tag="eechunk", name=f"ee{nm}")
            nc.scalar.activation(t[:, :, 0:s1 - s0, :], em_t[:, :, 0:s1 - s0, :],
                                 Exp, bias=dbias)
            return t

        # ---- chain state ----
        cf = 0                       # forward chunk index
        cb = (n_steps - 1) // W      # backward chunk index
        em_f = load_chunk(cf, emf, "f0")
        em_b = load_chunk(cb, emb, "b0") if cb != cf else em_f
        ee_f = exp_chunk(cf, em_f, eef, "f0")
        ee_b = exp_chunk(cb, em_b, eeb, "b0") if cb != cf else ee_f

        p = []   # forward states per group
        u = []   # backward states per group
        for g in range(G):
            pt = ppool.tile([128, 2, GB], bf16, tag=f"pf{g}", name=f"pf{g}")
            ut = ppool.tile([128, 2, GB], bf16, tag=f"pb{g}", name=f"pb{g}")
            for i in range(2):
                nc.scalar.activation(pt[:, i, :],
                                     em_f[:, i, 0, g * GB:(g + 1) * GB],
                                     Exp, bias=st_t[i])
                nc.scalar.activation(ut[:, i, :],
                                     em_b[:, i, (n_steps - 1) % W,
                                          g * GB:(g + 1) * GB],
                                     Exp, bias=ben[i])
            p.append(pt)
            u.append(ut)

        recf = [None] * G
        recb = [None] * G

        def chain_round(g, state, Emat, qtag, ee_t, w, rec, nm):
            """One MM+DVE round for one chain; returns (new_state, q tiles)."""
            q0 = qpool.tile([128, GB], f32, tag=f"{qtag}0", name=f"{qtag}0")
            q1 = qpool.tile([128, GB], f32, tag=f"{qtag}1", name=f"{qtag}1")
            for j, qj in enumerate((q0, q1)):
                for i in range(2):
                    nc.tensor.matmul(qj, Emat[i][j], state[:, i, :],
                                     start=(i == 0), stop=(i == 1))
            newt = ppool.tile([128, 2, GB], bf16, tag=nm, name=nm)
            for j, qj in enumerate((q0, q1)):
                eesl = ee_t[:, j, w, g * GB:(g + 1) * GB]
                if rec is not None:
                    ee2 = rpool.tile([128, GB], bf16, tag=f"sc{nm}{j}",
                                     name=f"sc{nm}{j}")
                    nc.vector.tensor_mul(ee2, eesl, rec)
                    eesl = ee2
                nc.vector.tensor_mul(newt[:, j, :], qj, eesl)
            return newt

        def renorm(g, state, d):
            sp = spool.tile([128, GB], f32, tag="rsum", name=f"rsum{d}{g}")
            for i in range(2):
                nc.tensor.matmul(sp, ones, state[:, i, :],
                                 start=(i == 0), stop=(i == 1))
            rc = rpool.tile([128, GB], f32, tag=f"rc{d}{g}", name=f"rc{d}{g}")
            nc.vector.reciprocal(rc, sp)
            lg = rpool.tile([1, GB], f32, tag=f"lg{d}{g}", name=f"lg{d}{g}")
            nc.scalar.activation(lg, sp[0:1, :], Ln)
            nc.vector.tensor_add(accs[(d, g)], accs[(d, g)], lg)
            return rc

        n_rounds = max(Rf, Rb)
        for r in range(1, n_rounds + 1):
            sf = r                     # forward step index (uses ee_sf)
            sb = n_steps - 1 - r       # backward: produces u_sb using ee_sb
            if sf <= Rf and sf // W != cf:
                cf = sf // W
                em_f = load_chunk(cf, emf, f"f{cf}")
                ee_f = exp_chunk(cf, em_f, eef, f"f{cf}")
            if sb >= Rf + 1 and sb // W != cb:
                cb = sb // W
                em_b = load_chunk(cb, emb, f"b{cb}")
                ee_b = exp_chunk(cb, em_b, eeb, f"b{cb}")
            for g in range(G):
                if sf <= Rf:
                    p[g] = chain_round(g, p[g], E, f"qf{g}", ee_f, sf % W,
                                       recf[g], f"pf{g}")
                    recf[g] = None
                if sb >= Rf + 1:
                    u[g] = chain_round(g, u[g], Et, f"qb{g}", ee_b, sb % W,
                                       recb[g], f"pb{g}")
                    recb[g] = None
            if r % R == 0:
                for g in range(G):
                    if sf < Rf:
                        recf[g] = renorm(g, p[g], "f")
                    if sb > Rf + 1:
                        recb[g] = renorm(g, u[g], "b")

        # ---- final: Z = (E^T alpha_Rf)^T u_{Rf+1} ----
        for g in range(G):
            q0 = qpool.tile([128, GB], f32, tag=f"qf{g}0", name=f"qfin{g}0")
            q1 = qpool.tile([128, GB], f32, tag=f"qf{g}1", name=f"qfin{g}1")
            for j, qj in enumerate((q0, q1)):
                for i in range(2):
                    nc.tensor.matmul(qj, E[i][j], p[g][:, i, :],
                                     start=(i == 0), stop=(i == 1))
            d = rpool.tile([128, 2, GB], f32, tag=f"d{g}", name=f"d{g}")
            nc.vector.tensor_mul(d[:, 0, :], q0, u[g][:, 0, :])
            nc.vector.tensor_mul(d[:, 1, :], q1, u[g][:, 1, :])
            fin = spool.tile([1, GB], f32, tag="fin", name=f"fin{g}")
            for i in range(2):
                nc.tensor.matmul(fin, onesf, d[:, i, :],
                                 start=(i == 0), stop=(i == 1))
            fl = rpool.tile([1, GB], f32, tag=f"fl{g}", name=f"fl{g}")
            nc.scalar.activation(fl, fin, Ln)
            res = rpool.tile([1, GB], f32, tag=f"res{g}", name=f"res{g}")
            nc.vector.tensor_add(res, fl, accs[("f", g)])
            res2 = rpool.tile([1, GB], f32, tag=f"res2{g}", name=f"res2{g}")
            nc.vector.tensor_add(res2, res, accs[("b", g)])
            nc.sync.dma_start(out=out[0:1, g * GB:(g + 1) * GB], in_=res2)

    if KEEP_MM_WAITS:
        nc.move_matmul_waits_to_ldweights = lambda: None
    nc.compile()
    return nc


def _prep_inputs(emissions, transitions, start_transitions, end_transitions,
                 n_steps=S):
    """Host-side layout prep: per-core input maps."""
    emissions = np.ascontiguousarray(emissions[:, :n_steps, :], dtype=np.float32)
    em_t = np.ascontiguousarray(emissions.transpose(2, 1, 0)).reshape(
        2, 128, n_steps, B)  # [i, p, s, b]
    trm = np.asarray(transitions, np.float32)
    tr = np.ascontiguousarray(trm.reshape(2, 128, 2, 128))
    trt = np.ascontiguousarray(trm.T.reshape(2, 128, 2, 128))
    stw = np.ascontiguousarray(
        np.asarray(start_transitions, np.float32).reshape(2, 128, 1))
    enw = np.ascontiguousarray(
        np.asarray(end_transitions, np.float32).reshape(2, 128, 1))
    in_maps = []
    for c in range(NCORES):
        in_maps.append({
            "em": np.ascontiguousarray(em_t[:, :, :, c * BPC:(c + 1) * BPC]),
            "tr": tr, "trt": trt, "stw": stw, "enw": enw,
        })
    return in_maps


def _gold_score_host(emissions, tags, mask, transitions, start_transitions,
                     end_transitions):
    emissions = np.asarray(emissions, np.float32)
    tags = np.asarray(tags, np.int64)
    m = np.asarray(mask, np.float32)
    emit = np.take_along_axis(emissions, tags[..., None], axis=2)[..., 0]
    trans = np.asarray(transitions, np.float32)[tags[:, :-1], tags[:, 1:]]
    score = (np.asarray(start_transitions, np.float32)[tags[:, 0]] + emit[:, 0]
             + ((emit[:, 1:] + trans) * m[:, 1:]).sum(axis=1))
    last_idx = np.asarray(mask, np.int64).sum(axis=1) - 1
    last_tags = np.take_along_axis(tags, last_idx[:, None], axis=1)[:, 0]
    return score + np.asarray(end_transitions, np.float32)[last_tags]


def _numpy_fallback(emissions, tags, mask, transitions, start_transitions,
                    end_transitions):
    """Reference-faithful numpy path (only used if mask is not all ones)."""
    em = np.asarray(emissions, np.float64)
    msk = np.asarray(mask, bool)
    trn = np.asarray(transitions, np.float64)
    alpha = np.asarray(start_transitions, np.float64)[None, :] + em[:, 0]
    for s in range(1, em.shape[1]):
        scores = alpha[:, :, None] + trn[None, :, :] + em[:, s][:, None, :]
        mx = scores.max(axis=1, keepdims=True)
        new = np.log(np.exp(scores - mx).sum(axis=1)) + mx[:, 0, :]
        alpha = np.where(msk[:, s][:, None], new, alpha)
    fin = alpha + np.asarray(end_transitions, np.float64)[None, :]
    mx = fin.max(axis=1, keepdims=True)
    logden = np.log(np.exp(fin - mx).sum(axis=1)) + mx[:, 0]
    gold = _gold_score_host(emissions, tags, mask, transitions,
                            start_transitions, end_transitions)
    return np.array(np.mean(gold - logden), dtype=np.float32)


def run_device(emissions, transitions, start_transitions, end_transitions,
               n_steps=S, trace=False, tmpdir=None):
    """Compile (cached) + run the Bass kernel; returns (logden[B], results_obj)."""
    from concourse.bass_utils import run_bass_kernel_spmd
    key = n_steps
    if key not in _cache:
        _cache[key] = build_nc(n_steps)
    nc = _cache[key]
    in_maps = _prep_inputs(emissions, transitions, start_transitions,
                           end_transitions, n_steps)
    core_ids = list(range(NCORES))
    r = run_bass_kernel_spmd(nc, in_maps, core_ids, trace=trace, tmpdir=tmpdir)
    logden = np.concatenate([np.asarray(r.results[c]["out"][0], np.float32)
                             for c in range(NCORES)])
    logden = logden + np.float32((n_steps - 1) * DELTA)
    return logden, r


def kernel(emissions, tags, mask, transitions, start_transitions,
           end_transitions):
    emissions = np.asarray(emissions)
    tags = np.asarray(tags)
    mask = np.asarray(mask)
    if not mask.all():
        return _numpy_fallback(emissions, tags, mask, transitions,
                               start_transitions, end_transitions)
    logden, _ = run_device(emissions, transitions, start_transitions,
                           end_transitions)
    gold = _gold_score_host(emissions, tags, mask, transitions,
                            start_transitions, end_transitions)
    return np.array(np.mean(gold - logden), dtype=np.float32)


# revision 13
# speedup vs baseline: 1.8791x; 1.0127x over previous
"""Trainium2 Bass kernel for CRF mean log-likelihood (B=128, S=512, T=256).

Strategy: data-parallel over batch (16 sequences per core, 8 cores). The
forward-algorithm log-partition is computed in exponential space so the
per-step T x T logsumexp becomes a PE matmul:

    alpha_s = (E^T alpha_{s-1}) * exp(emit_s - delta)     E = exp(trans)

with a constant per-step shift delta ~= log(T) + 1/2 (keeps the state in a
narrow dynamic range; validated drift < +-6 in log space) and an exact
renormalization every R steps for safety (sum via ones-matmul, folded into
the next step's emission factor, off the critical path).

The chain is latency-bound (matmul -> DVE multiply -> matmul), so the
sequence is processed FROM BOTH ENDS simultaneously (meet in the middle):
  forward:  alpha_s = (E^T alpha_{s-1}) . ee_s          s = 1..Rf
  backward: u_s = (E u_{s+1}) . ee_s   (u_s=gamma_s.ee_s), s = S-2..Rf+1
  Z        = (E^T alpha_Rf)^T  u_{Rf+1}
Two independent chains per batch group halve the sequential depth.

The gold (numerator) score is O(B*S) gather work — computed on host.
"""
import numpy as np

B, S, T = 128, 512, 256
NCORES = 8
BPC = B // NCORES          # batch per core = 16
G = 1                      # batch groups per core (chains = 2*G)
GB = BPC // G
W = 128                    # steps per emissions chunk
R = 64                     # renormalization period
DELTA = 6.045              # per-step log-space shift ~ log(256) + 0.5
KEEP_MM_WAITS = True       # skip bacc's move_matmul_waits_to_ldweights

_cache = {}


def build_nc(n_steps=S):
    import concourse.bass as bass
    import concourse.tile as tile
    from concourse import bacc, mybir
    from contextlib import ExitStack

    f32 = mybir.dt.float32
    bf16 = mybir.dt.bfloat16
    Exp = mybir.ActivationFunctionType.Exp
    Ln = mybir.ActivationFunctionType.Ln

    assert n_steps >= 4
    Rf = (n_steps - 2) // 2          # forward DVE-rounds (alpha_1..alpha_Rf)
    Rb = n_steps - 2 - Rf            # backward rounds (u_{S-2}..u_{Rf+1})

    nc = bacc.Bacc()
    em = nc.declare_dram_parameter("em", [2, 128, n_steps, BPC], f32, isOutput=False)
    tr = nc.declare_dram_parameter("tr", [2, 128, 2, 128], f32, isOutput=False)
    trt = nc.declare_dram_parameter("trt", [2, 128, 2, 128], f32, isOutput=False)
    stw = nc.declare_dram_parameter("stw", [2, 128, 1], f32, isOutput=False)
    enw = nc.declare_dram_parameter("enw", [2, 128, 1], f32, isOutput=False)
    out = nc.declare_dram_parameter("out", [1, BPC], f32, isOutput=True)

    with ExitStack() as ctx:
        tc = ctx.enter_context(tile.TileContext(nc))
        const = ctx.enter_context(tc.tile_pool(name="const", bufs=1))
        emf = ctx.enter_context(tc.tile_pool(name="emf", bufs=2))
        eef = ctx.enter_context(tc.tile_pool(name="eef", bufs=2))
        emb = ctx.enter_context(tc.tile_pool(name="emb", bufs=2))
        eeb = ctx.enter_context(tc.tile_pool(name="eeb", bufs=2))
        ppool = ctx.enter_context(tc.tile_pool(name="p", bufs=3))
        rpool = ctx.enter_context(tc.tile_pool(name="rn", bufs=2))
        qpool = ctx.enter_context(tc.tile_pool(name="q", bufs=1, space="PSUM"))
        spool = ctx.enter_context(tc.tile_pool(name="s", bufs=2, space="PSUM"))

        # ---- one-time constants ----
        E = [[None, None], [None, None]]   # E[i][j]: lhsT for forward
        Et = [[None, None], [None, None]]  # Et[i][j]: lhsT for backward
        for i in range(2):
            stage = rpool.tile([128, 2, 128], f32, tag="trstage", name="trstage")
            nc.sync.dma_start(out=stage, in_=tr[i])
            for j in range(2):
                E[i][j] = const.tile([128, 128], bf16, tag=f"E{i}{j}", name=f"E{i}{j}")
                nc.scalar.activation(E[i][j], stage[:, j, :], Exp)
        for i in range(2):
            stage = rpool.tile([128, 2, 128], f32, tag="trstage", name="trstaget")
            nc.sync.dma_start(out=stage, in_=trt[i])
            for j in range(2):
                Et[i][j] = const.tile([128, 128], bf16, tag=f"Et{i}{j}",
                                      name=f"Et{i}{j}")
                nc.scalar.activation(Et[i][j], stage[:, j, :], Exp)
        ones = const.tile([128, 128], bf16, tag="ones", name="ones")
        nc.vector.memset(ones, 1.0)
        onesf = const.tile([128, 1], f32, tag="onesf", name="onesf")
        nc.vector.memset(onesf, 1.0)
        dbias = const.tile([128, 1], f32, tag="dbias", name="dbias")
        nc.vector.memset(dbias, -DELTA)
        st_t = []
        for i in range(2):
            t = const.tile([128, 1], f32, tag=f"st{i}", name=f"st{i}")
            nc.sync.dma_start(out=t, in_=stw[i])
            st_t.append(t)
        ben = []
        for i in range(2):
            stage = rpool.tile([128, 1], f32, tag="enstage", name="enstage")
            nc.sync.dma_start(out=stage, in_=enw[i])
            t = const.tile([128, 1], f32, tag=f"ben{i}", name=f"ben{i}")
            nc.vector.tensor_add(t, stage, dbias)   # end - delta (bwd init bias)
            ben.append(t)
        accs = {}
        for d in ("f", "b"):
            for g in range(G):
                a = const.tile([1, GB], f32, tag=f"acc{d}{g}", name=f"acc{d}{g}")
                nc.vector.memset(a, 1.0)
                accs[(d, g)] = a

        # ---- emissions chunk streaming (per direction) ----
        def load_chunk(c, pool, nm):
            s0, s1 = c * W, min(n_steps, (c + 1) * W)
            t = pool.tile([128, 2, W, BPC], f32, tag="emchunk", name=f"em{nm}")
            for i in range(2):
                nc.sync.dma_start(out=t[:, i, 0:s1 - s0, :], in_=em[i, :, s0:s1, :])
            return t

        def exp_chunk(c, em_t, pool, nm):
            s0, s1 = c * W, min(n_steps, (c + 1) * W)
            t = pool.tile([128, 2, W, BPC], bf16, tag="eechunk", name=f"ee{nm}")
            n = s1 - s0
            nsub = 4 if n >= 8 else 1
            for k in range(nsub):
                a, b = k * n // nsub, (k + 1) * n // nsub
                nc.scalar.activation(t[:, :, a:b, :], em_t[:, :, a:b, :],
                                     Exp, bias=dbias)
            return t

        # ---- chain state ----
        cf = 0                       # forward chunk index
        cb = (n_steps - 1) // W      # backward chunk index
        em_f = load_chunk(cf, emf, "f0")
        em_b = load_chunk(cb, emb, "b0") if cb != cf else em_f
        ee_f = exp_chunk(cf, em_f, eef, "f0")
        ee_b = exp_chunk(cb, em_b, eeb, "b0") if cb != cf else ee_f

        p = []   # forward states per group
        u = []   # backward states per group
        for g in range(G):
            pt = ppool.tile([128, 2, GB], bf16, tag=f"pf{g}", name=f"pf{g}")
            ut = ppool.tile([128, 2, GB], bf16, tag=f"pb{g}", name=f"pb{g}")
            for i in range(2):
                nc.scalar.activation(pt[:, i, :],
                                     em_f[:, i, 0, g * GB:(g + 1) * GB],
                                     Exp, bias=st_t[i])
                nc.scalar.activation(ut[:, i, :],
                                     em_b[:, i, (n_steps - 1) % W,
                                          g * GB:(g + 1) * GB],
                                     Exp, bias=ben[i])
            p.append(pt)
            u.append(ut)

        recf = [None] * G
        recb = [None] * G

        def chain_round(g, state, Emat, qtag, ee_t, w, rec, nm):
            """One MM+DVE round for one chain; returns (new_state, q tiles)."""
            q0 = qpool.tile([128, GB], f32, tag=f"{qtag}0", name=f"{qtag}0")
            q1 = qpool.tile([128, GB], f32, tag=f"{qtag}1", name=f"{qtag}1")
            for j, qj in enumerate((q0, q1)):
                for i in range(2):
                    nc.tensor.matmul(qj, Emat[i][j], state[:, i, :],
                                     start=(i == 0), stop=(i == 1))
            newt = ppool.tile([128, 2, GB], bf16, tag=nm, name=nm)
            for j, qj in enumerate((q0, q1)):
                eesl = ee_t[:, j, w, g * GB:(g + 1) * GB]
                if rec is not None:
                    ee2 = rpool.tile([128, GB], bf16, tag=f"sc{nm}{j}",
                                     name=f"sc{nm}{j}")
                    nc.vector.tensor_mul(ee2, eesl, rec)
                    eesl = ee2
                nc.vector.tensor_mul(newt[:, j, :], qj, eesl)
            return newt

        def renorm(g, state, d):
            sp = spool.tile([128, GB], f32, tag="rsum", name=f"rsum{d}{g}")
            for i in range(2):
                nc.tensor.matmul(sp, ones, state[:, i, :],
                                 start=(i == 0), stop=(i == 1))
            rc = rpool.tile([128, GB], f32, tag=f"rc{d}{g}", name=f"rc{d}{g}")
            nc.vector.reciprocal(rc, sp)
            nc.vector.tensor_mul(accs[(d, g)], accs[(d, g)], sp[0:1, :])
            return rc

        n_rounds = max(Rf, Rb)
        for r in range(1, n_rounds + 1):
            sf = r                     # forward step index (uses ee_sf)
            sb = n_steps - 1 - r       # backward: produces u_sb using ee_sb
            if sf <= Rf and sf // W != cf:
                cf = sf // W
                em_f = load_chunk(cf, emf, f"f{cf}")
                ee_f = exp_chunk(cf, em_f, eef, f"f{cf}")
            if sb >= Rf + 1 and sb // W != cb:
                cb = sb // W
                em_b = load_chunk(cb, emb, f"b{cb}")
                ee_b = exp_chunk(cb, em_b, eeb, f"b{cb}")
            for g in range(G):
                if sf <= Rf:
                    p[g] = chain_round(g, p[g], E, f"qf{g}", ee_f, sf % W,
                                       recf[g], f"pf{g}")
                    recf[g] = None
                if sb >= Rf + 1:
                    u[g] = chain_round(g, u[g], Et, f"qb{g}", ee_b, sb % W,
                                       recb[g], f"pb{g}")
                    recb[g] = None
            if r % R == 0:
                for g in range(G):
                    if sf < Rf:
                        recf[g] = renorm(g, p[g], "f")
                    if sb > Rf + 1:
                        recb[g] = renorm(g, u[g], "b")

        # ---- final: Z = (E^T alpha_Rf)^T u_{Rf+1} ----
        for g in range(G):
            q0 = qpool.tile([128, GB], f32, tag=f"qf{g}0", name=f"qfin{g}0")
            q1 = qpool.tile([128, GB], f32, tag=f"qf{g}1", name=f"qfin{g}1")
            for j, qj in enumerate((q0, q1)):
                for i in range(2):
                    nc.tensor.matmul(qj, E[i][j], p[g][:, i, :],
                                     start=(i == 0), stop=(i == 1))
            d = rpool.tile([128, 2, GB], f32, tag=f"d{g}", name=f"d{g}")
            nc.vector.tensor_mul(d[:, 0, :], q0, u[g][:, 0, :])
            nc.vector.tensor_mul(d[:, 1, :], q1, u[g][:, 1, :])
            fin = spool.tile([1, GB], f32, tag="fin", name=f"fin{g}")
            for i in range(2):
                nc.tensor.matmul(fin, onesf, d[:, i, :],
                                 start=(i == 0), stop=(i == 1))
            res = rpool.tile([1, GB], f32, tag=f"res{g}", name=f"res{g}")
            nc.vector.tensor_mul(res, fin, accs[("f", g)])
            res2 = rpool.tile([1, GB], f32, tag=f"res2{g}", name=f"res2{g}")
            nc.vector.tensor_mul(res2, res, accs[("b", g)])
            nc.sync.dma_start(out=out[0:1, g * GB:(g + 1) * GB], in_=res2)

    if KEEP_MM_WAITS:
        nc.move_matmul_waits_to_ldweights = lambda: None
    nc.compile()
    return nc


def _prep_inputs(emissions, transitions, start_transitions, end_transitions,
                 n_steps=S):
    """Host-side layout prep: per-core input maps."""
    emissions = np.ascontiguousarray(emissions[:, :n_steps, :], dtype=np.float32)
    em_t = np.ascontiguousarray(emissions.transpose(2, 1, 0)).reshape(
        2, 128, n_steps, B)  # [i, p, s, b]
    trm = np.asarray(transitions, np.float32)
    tr = np.ascontiguousarray(trm.reshape(2, 128, 2, 128))
    trt = np.ascontiguousarray(trm.T.reshape(2, 128, 2, 128))
    stw = np.ascontiguousarray(
        np.asarray(start_transitions, np.float32).reshape(2, 128, 1))
    enw = np.ascontiguousarray(
        np.asarray(end_transitions, np.float32).reshape(2, 128, 1))
    in_maps = []
    for c in range(NCORES):
        in_maps.append({
            "em": np.ascontiguousarray(em_t[:, :, :, c * BPC:(c + 1) * BPC]),
            "tr": tr, "trt": trt, "stw": stw, "enw": enw,
        })
    return in_maps


def _gold_score_host(emissions, tags, mask, transitions, start_transitions,
                     end_transitions):
    emissions = np.asarray(emissions, np.float32)
    tags = np.asarray(tags, np.int64)
    m = np.asarray(mask, np.float32)
    emit = np.take_along_axis(emissions, tags[..., None], axis=2)[..., 0]
    trans = np.asarray(transitions, np.float32)[tags[:, :-1], tags[:, 1:]]
    score = (np.asarray(start_transitions, np.float32)[tags[:, 0]] + emit[:, 0]
             + ((emit[:, 1:] + trans) * m[:, 1:]).sum(axis=1))
    last_idx = np.asarray(mask, np.int64).sum(axis=1) - 1
    last_tags = np.take_along_axis(tags, last_idx[:, None], axis=1)[:, 0]
    return score + np.asarray(end_transitions, np.float32)[last_tags]


def _numpy_fallback(emissions, tags, mask, transitions, start_transitions,
                    end_transitions):
    """Reference-faithful numpy path (only used if mask is not all ones)."""
    em = np.asarray(emissions, np.float64)
    msk = np.asarray(mask, bool)
    trn = np.asarray(transitions, np.float64)
    alpha = np.asarray(start_transitions, np.float64)[None, :] + em[:, 0]
    for s in range(1, em.shape[1]):
        scores = alpha[:, :, None] + trn[None, :, :] + em[:, s][:, None, :]
        mx = scores.max(axis=1, keepdims=True)
        new = np.log(np.exp(scores - mx).sum(axis=1)) + mx[:, 0, :]
        alpha = np.where(msk[:, s][:, None], new, alpha)
    fin = alpha + np.asarray(end_transitions, np.float64)[None, :]
    mx = fin.max(axis=1, keepdims=True)
    logden = np.log(np.exp(fin - mx).sum(axis=1)) + mx[:, 0]
    gold = _gold_score_host(emissions, tags, mask, transitions,
                            start_transitions, end_transitions)
    return np.array(np.mean(gold - logden), dtype=np.float32)


def run_device(emissions, transitions, start_transitions, end_transitions,
               n_steps=S, trace=False, tmpdir=None):
    """Compile (cached) + run the Bass kernel; returns (logden[B], results_obj)."""
    from concourse.bass_utils import run_bass_kernel_spmd
    key = n_steps
    if key not in _cache:
        _cache[key] = build_nc(n_steps)
    nc = _cache[key]
    in_maps = _prep_inputs(emissions, transitions, start_transitions,
                           end_transitions, n_steps)
    core_ids = list(range(NCORES))
    r = run_bass_kernel_spmd(nc, in_maps, core_ids, trace=trace, tmpdir=tmpdir)
    zprod = np.concatenate([np.asarray(r.results[c]["out"][0], np.float32)
                            for c in range(NCORES)])
    logden = np.log(zprod) + np.float32((n_steps - 1) * DELTA)
    return logden, r


def kernel(emissions, tags, mask, transitions, start_transitions,
           end_transitions):
    emissions = np.asarray(emissions)
    tags = np.asarray(tags)
    mask = np.asarray(mask)
    if not mask.all():
        return _numpy_fallback(emissions, tags, mask, transitions,
                               start_transitions, end_transitions)
    logden, _ = run_device(emissions, transitions, start_transitions,
                           end_transitions)
    gold = _gold_score_host(emissions, tags, mask, transitions,
                            start_transitions, end_transitions)
    return np.array(np.mean(gold - logden), dtype=np.float32)


# revision 14
# speedup vs baseline: 1.9370x; 1.0308x over previous
"""Trainium2 Bass kernel for CRF mean log-likelihood (B=128, S=512, T=256).

Strategy: data-parallel over batch (16 sequences per core, 8 cores). The
forward-algorithm log-partition is computed in exponential space so the
per-step T x T logsumexp becomes a PE matmul:

    alpha_s = (E^T alpha_{s-1}) * exp(emit_s - delta)     E = exp(trans)

with a constant per-step shift delta ~= log(T) + 1/2 (keeps the state in a
narrow dynamic range; validated drift < +-6 in log space) and an exact
renormalization every R steps for safety (sum via ones-matmul, folded into
the next step's emission factor, off the critical path).

The chain is latency-bound (matmul -> DVE multiply -> matmul), so the
sequence is processed FROM BOTH ENDS simultaneously (meet in the middle):
  forward:  alpha_s = (E^T alpha_{s-1}) . ee_s          s = 1..Rf
  backward: u_s = (E u_{s+1}) . ee_s   (u_s=gamma_s.ee_s), s = S-2..Rf+1
  Z        = (E^T alpha_Rf)^T  u_{Rf+1}
Two independent chains per batch group halve the sequential depth.

The gold (numerator) score is O(B*S) gather work — computed on host.
"""
import numpy as np

B, S, T = 128, 512, 256
NCORES = 8
BPC = B // NCORES          # batch per core = 16
G = 1                      # batch groups per core (chains = 2*G)
GB = BPC // G
W = 128                    # steps per emissions chunk
R = 0                      # renormalization period (0 = off; drift is ~+-6 nats over a chain, far within f32 range)
DELTA = 6.045              # per-step log-space shift ~ log(256) + 0.5
KEEP_MM_WAITS = True       # skip bacc's move_matmul_waits_to_ldweights

_cache = {}


def build_nc(n_steps=S):
    import concourse.bass as bass
    import concourse.tile as tile
    from concourse import bacc, mybir
    from contextlib import ExitStack

    f32 = mybir.dt.float32
    bf16 = mybir.dt.bfloat16
    Exp = mybir.ActivationFunctionType.Exp
    Ln = mybir.ActivationFunctionType.Ln

    assert n_steps >= 4
    Rf = (n_steps - 2) // 2          # forward DVE-rounds (alpha_1..alpha_Rf)
    Rb = n_steps - 2 - Rf            # backward rounds (u_{S-2}..u_{Rf+1})

    nc = bacc.Bacc()
    em = nc.declare_dram_parameter("em", [2, 128, n_steps, BPC], f32, isOutput=False)
    tr = nc.declare_dram_parameter("tr", [2, 128, 2, 128], f32, isOutput=False)
    trt = nc.declare_dram_parameter("trt", [2, 128, 2, 128], f32, isOutput=False)
    stw = nc.declare_dram_parameter("stw", [2, 128, 1], f32, isOutput=False)
    enw = nc.declare_dram_parameter("enw", [2, 128, 1], f32, isOutput=False)
    out = nc.declare_dram_parameter("out", [1, BPC], f32, isOutput=True)

    with ExitStack() as ctx:
        tc = ctx.enter_context(tile.TileContext(nc))
        const = ctx.enter_context(tc.tile_pool(name="const", bufs=1))
        emf = ctx.enter_context(tc.tile_pool(name="emf", bufs=2))
        eef = ctx.enter_context(tc.tile_pool(name="eef", bufs=2))
        emb = ctx.enter_context(tc.tile_pool(name="emb", bufs=2))
        eeb = ctx.enter_context(tc.tile_pool(name="eeb", bufs=2))
        ppool = ctx.enter_context(tc.tile_pool(name="p", bufs=3))
        rpool = ctx.enter_context(tc.tile_pool(name="rn", bufs=2))
        qpool = ctx.enter_context(tc.tile_pool(name="q", bufs=1, space="PSUM"))
        spool = ctx.enter_context(tc.tile_pool(name="s", bufs=2, space="PSUM"))

        # ---- one-time constants ----
        E = [[None, None], [None, None]]   # E[i][j]: lhsT for forward
        Et = [[None, None], [None, None]]  # Et[i][j]: lhsT for backward
        for i in range(2):
            stage = rpool.tile([128, 2, 128], f32, tag="trstage", name="trstage")
            nc.sync.dma_start(out=stage, in_=tr[i])
            for j in range(2):
                E[i][j] = const.tile([128, 128], bf16, tag=f"E{i}{j}", name=f"E{i}{j}")
                nc.scalar.activation(E[i][j], stage[:, j, :], Exp)
        for i in range(2):
            stage = rpool.tile([128, 2, 128], f32, tag="trstage", name="trstaget")
            nc.sync.dma_start(out=stage, in_=trt[i])
            for j in range(2):
                Et[i][j] = const.tile([128, 128], bf16, tag=f"Et{i}{j}",
                                      name=f"Et{i}{j}")
                nc.scalar.activation(Et[i][j], stage[:, j, :], Exp)
        ones = const.tile([128, 128], bf16, tag="ones", name="ones")
        nc.vector.memset(ones, 1.0)
        onesf = const.tile([128, 1], f32, tag="onesf", name="onesf")
        nc.vector.memset(onesf, 1.0)
        dbias = const.tile([128, 1], f32, tag="dbias", name="dbias")
        nc.vector.memset(dbias, -DELTA)
        st_t = []
        for i in range(2):
            t = const.tile([128, 1], f32, tag=f"st{i}", name=f"st{i}")
            nc.sync.dma_start(out=t, in_=stw[i])
            st_t.append(t)
        ben = []
        for i in range(2):
            stage = rpool.tile([128, 1], f32, tag="enstage", name="enstage")
            nc.sync.dma_start(out=stage, in_=enw[i])
            t = const.tile([128, 1], f32, tag=f"ben{i}", name=f"ben{i}")
            nc.vector.tensor_add(t, stage, dbias)   # end - delta (bwd init bias)
            ben.append(t)
        accs = {}
        for d in ("f", "b"):
            for g in range(G):
                a = const.tile([1, GB], f32, tag=f"acc{d}{g}", name=f"acc{d}{g}")
                nc.vector.memset(a, 1.0)
                accs[(d, g)] = a

        # ---- emissions chunk streaming (per direction) ----
        # Stream each chunk in 16-step pieces (DMA pair + exp ACT per piece),
        # ordered by consumption direction, so the first rounds' ee slices are
        # ready within ~1us of kernel start instead of after the full chunk.
        def load_chunk(c, pool, eepool_, nm, descending=False):
            s0, s1 = c * W, min(n_steps, (c + 1) * W)
            n = s1 - s0
            t = pool.tile([128, 2, W, BPC], f32, tag="emchunk", name=f"em{nm}")
            te = eepool_.tile([128, 2, W, BPC], bf16, tag="eechunk",
                              name=f"ee{nm}")
            pieces = [(a, min(a + 16, n)) for a in range(0, n, 16)]
            if descending:
                pieces = pieces[::-1]
            for a, b in pieces:
                for i in range(2):
                    nc.sync.dma_start(out=t[:, i, a:b, :],
                                      in_=em[i, :, s0 + a:s0 + b, :])
                nc.scalar.activation(te[:, :, a:b, :], t[:, :, a:b, :],
                                     Exp, bias=dbias)
            return t, te

        # ---- chain state ----
        cf = 0                       # forward chunk index
        cb = (n_steps - 1) // W      # backward chunk index
        em_f, ee_f = load_chunk(cf, emf, eef, "f0")
        if cb != cf:
            em_b, ee_b = load_chunk(cb, emb, eeb, "b0", descending=True)
        else:
            em_b, ee_b = em_f, ee_f

        p = []   # forward states per group
        u = []   # backward states per group
        for g in range(G):
            pt = ppool.tile([128, 2, GB], bf16, tag=f"pf{g}", name=f"pf{g}")
            ut = ppool.tile([128, 2, GB], bf16, tag=f"pb{g}", name=f"pb{g}")
            for i in range(2):
                nc.scalar.activation(pt[:, i, :],
                                     em_f[:, i, 0, g * GB:(g + 1) * GB],
                                     Exp, bias=st_t[i])
                nc.scalar.activation(ut[:, i, :],
                                     em_b[:, i, (n_steps - 1) % W,
                                          g * GB:(g + 1) * GB],
                                     Exp, bias=ben[i])
            p.append(pt)
            u.append(ut)

        recf = [None] * G
        recb = [None] * G

        def chain_round(g, state, Emat, qtag, ee_t, w, rec, nm):
            """One MM+DVE round for one chain; returns (new_state, q tiles)."""
            q0 = qpool.tile([128, GB], f32, tag=f"{qtag}0", name=f"{qtag}0")
            q1 = qpool.tile([128, GB], f32, tag=f"{qtag}1", name=f"{qtag}1")
            for j, qj in enumerate((q0, q1)):
                for i in range(2):
                    nc.tensor.matmul(qj, Emat[i][j], state[:, i, :],
                                     start=(i == 0), stop=(i == 1))
            newt = ppool.tile([128, 2, GB], bf16, tag=nm, name=nm)
            for j, qj in enumerate((q0, q1)):
                eesl = ee_t[:, j, w, g * GB:(g + 1) * GB]
                if rec is not None:
                    ee2 = rpool.tile([128, GB], bf16, tag=f"sc{nm}{j}",
                                     name=f"sc{nm}{j}")
                    nc.vector.tensor_mul(ee2, eesl, rec)
                    eesl = ee2
                nc.vector.tensor_mul(newt[:, j, :], qj, eesl)
            return newt

        def renorm(g, state, d):
            sp = spool.tile([128, GB], f32, tag="rsum", name=f"rsum{d}{g}")
            for i in range(2):
                nc.tensor.matmul(sp, ones, state[:, i, :],
                                 start=(i == 0), stop=(i == 1))
            rc = rpool.tile([128, GB], f32, tag=f"rc{d}{g}", name=f"rc{d}{g}")
            nc.vector.reciprocal(rc, sp)
            nc.vector.tensor_mul(accs[(d, g)], accs[(d, g)], sp[0:1, :])
            return rc

        n_rounds = max(Rf, Rb)
        for r in range(1, n_rounds + 1):
            sf = r                     # forward step index (uses ee_sf)
            sb = n_steps - 1 - r       # backward: produces u_sb using ee_sb
            if sf <= Rf and sf // W != cf:
                cf = sf // W
                em_f, ee_f = load_chunk(cf, emf, eef, f"f{cf}")
            if sb >= Rf + 1 and sb // W != cb:
                cb = sb // W
                em_b, ee_b = load_chunk(cb, emb, eeb, f"b{cb}",
                                        descending=True)
            for g in range(G):
                if sf <= Rf:
                    p[g] = chain_round(g, p[g], E, f"qf{g}", ee_f, sf % W,
                                       recf[g], f"pf{g}")
                    recf[g] = None
                if sb >= Rf + 1:
                    u[g] = chain_round(g, u[g], Et, f"qb{g}", ee_b, sb % W,
                                       recb[g], f"pb{g}")
                    recb[g] = None
            if R and r % R == 0:
                for g in range(G):
                    if sf < Rf:
                        recf[g] = renorm(g, p[g], "f")
                    if sb > Rf + 1:
                        recb[g] = renorm(g, u[g], "b")

        # ---- final: Z = (E^T alpha_Rf)^T u_{Rf+1} ----
        for g in range(G):
            q0 = qpool.tile([128, GB], f32, tag=f"qf{g}0", name=f"qfin{g}0")
            q1 = qpool.tile([128, GB], f32, tag=f"qf{g}1", name=f"qfin{g}1")
            for j, qj in enumerate((q0, q1)):
                for i in range(2):
                    nc.tensor.matmul(qj, E[i][j], p[g][:, i, :],
                                     start=(i == 0), stop=(i == 1))
            d = rpool.tile([128, 2, GB], f32, tag=f"d{g}", name=f"d{g}")
            nc.vector.tensor_mul(d[:, 0, :], q0, u[g][:, 0, :])
            nc.vector.tensor_mul(d[:, 1, :], q1, u[g][:, 1, :])
            fin = spool.tile([1, GB], f32, tag="fin", name=f"fin{g}")
            for i in range(2):
                nc.tensor.matmul(fin, onesf, d[:, i, :],
                                 start=(i == 0), stop=(i == 1))
            res = rpool.tile([1, GB], f32, tag=f"res{g}", name=f"res{g}")
            nc.vector.tensor_mul(res, fin, accs[("f", g)])
            res2 = rpool.tile([1, GB], f32, tag=f"res2{g}", name=f"res2{g}")
            nc.vector.tensor_mul(res2, res, accs[("b", g)])
            nc.sync.dma_start(out=out[0:1, g * GB:(g + 1) * GB], in_=res2)

    if KEEP_MM_WAITS:
        nc.move_matmul_waits_to_ldweights = lambda: None
    nc.compile()
    return nc


def _prep_inputs(emissions, transitions, start_transitions, end_transitions,
                 n_steps=S):
    """Host-side layout prep: per-core input maps."""
    emissions = np.ascontiguousarray(emissions[:, :n_steps, :], dtype=np.float32)
    em_t = np.ascontiguousarray(emissions.transpose(2, 1, 0)).reshape(
        2, 128, n_steps, B)  # [i, p, s, b]
    trm = np.asarray(transitions, np.float32)
    tr = np.ascontiguousarray(trm.reshape(2, 128, 2, 128))
    trt = np.ascontiguousarray(trm.T.reshape(2, 128, 2, 128))
    stw = np.ascontiguousarray(
        np.asarray(start_transitions, np.float32).reshape(2, 128, 1))
    enw = np.ascontiguousarray(
        np.asarray(end_transitions, np.float32).reshape(2, 128, 1))
    in_maps = []
    for c in range(NCORES):
        in_maps.append({
            "em": np.ascontiguousarray(em_t[:, :, :, c * BPC:(c + 1) * BPC]),
            "tr": tr, "trt": trt, "stw": stw, "enw": enw,
        })
    return in_maps


def _gold_score_host(emissions, tags, mask, transitions, start_transitions,
                     end_transitions):
    emissions = np.asarray(emissions, np.float32)
    tags = np.asarray(tags, np.int64)
    m = np.asarray(mask, np.float32)
    emit = np.take_along_axis(emissions, tags[..., None], axis=2)[..., 0]
    trans = np.asarray(transitions, np.float32)[tags[:, :-1], tags[:, 1:]]
    score = (np.asarray(start_transitions, np.float32)[tags[:, 0]] + emit[:, 0]
             + ((emit[:, 1:] + trans) * m[:, 1:]).sum(axis=1))
    last_idx = np.asarray(mask, np.int64).sum(axis=1) - 1
    last_tags = np.take_along_axis(tags, last_idx[:, None], axis=1)[:, 0]
    return score + np.asarray(end_transitions, np.float32)[last_tags]


def _numpy_fallback(emissions, tags, mask, transitions, start_transitions,
                    end_transitions):
    """Reference-faithful numpy path (only used if mask is not all ones)."""
    em = np.asarray(emissions, np.float64)
    msk = np.asarray(mask, bool)
    trn = np.asarray(transitions, np.float64)
    alpha = np.asarray(start_transitions, np.float64)[None, :] + em[:, 0]
    for s in range(1, em.shape[1]):
        scores = alpha[:, :, None] + trn[None, :, :] + em[:, s][:, None, :]
        mx = scores.max(axis=1, keepdims=True)
        new = np.log(np.exp(scores - mx).sum(axis=1)) + mx[:, 0, :]
        alpha = np.where(msk[:, s][:, None], new, alpha)
    fin = alpha + np.asarray(end_transitions, np.float64)[None, :]
    mx = fin.max(axis=1, keepdims=True)
    logden = np.log(np.exp(fin - mx).sum(axis=1)) + mx[:, 0]
    gold = _gold_score_host(emissions, tags, mask, transitions,
                            start_transitions, end_transitions)
    return np.array(np.mean(gold - logden), dtype=np.float32)


def run_device(emissions, transitions, start_transitions, end_transitions,
               n_steps=S, trace=False, tmpdir=None):
    """Compile (cached) + run the Bass kernel; returns (logden[B], results_obj)."""
    from concourse.bass_utils import run_bass_kernel_spmd
    key = n_steps
    if key not in _cache:
        _cache[key] = build_nc(n_steps)
    nc = _cache[key]
    in_maps = _prep_inputs(emissions, transitions, start_transitions,
                           end_transitions, n_steps)
    core_ids = list(range(NCORES))
    r = run_bass_kernel_spmd(nc, in_maps, core_ids, trace=trace, tmpdir=tmpdir)
    zprod = np.concatenate([np.asarray(r.results[c]["out"][0], np.float32)
                            for c in range(NCORES)])
    logden = np.log(zprod) + np.float32((n_steps - 1) * DELTA)
    return logden, r


def kernel(emissions, tags, mask, transitions, start_transitions,
           end_transitions):
    emissions = np.asarray(emissions)
    tags = np.asarray(tags)
    mask = np.asarray(mask)
    if not mask.all():
        return _numpy_fallback(emissions, tags, mask, transitions,
                               start_transitions, end_transitions)
    logden, _ = run_device(emissions, transitions, start_transitions,
                           end_transitions)
    gold = _gold_score_host(emissions, tags, mask, transitions,
                            start_transitions, end_transitions)
    return np.array(np.mean(gold - logden), dtype=np.float32)


# revision 16
# speedup vs baseline: 2.0571x; 1.0620x over previous
"""Trainium2 Bass kernel for CRF mean log-likelihood (B=128, S=512, T=256).

Strategy: data-parallel over batch (16 sequences per core, 8 cores). The
forward-algorithm log-partition is computed in exponential space so the
per-step T x T logsumexp becomes a PE matmul:

    alpha_s = (E^T alpha_{s-1}) * exp(emit_s - delta)     E = exp(trans)

with a constant per-step shift delta ~= log(T) + 1/2 (keeps the state in a
narrow dynamic range; validated drift < +-6 in log space) and an exact
renormalization every R steps for safety (sum via ones-matmul, folded into
the next step's emission factor, off the critical path).

The chain is latency-bound (matmul -> DVE multiply -> matmul), so the
sequence is processed FROM BOTH ENDS simultaneously (meet in the middle):
  forward:  alpha_s = (E^T alpha_{s-1}) . ee_s          s = 1..Rf
  backward: u_s = (E u_{s+1}) . ee_s   (u_s=gamma_s.ee_s), s = S-2..Rf+1
  Z        = (E^T alpha_Rf)^T  u_{Rf+1}
Two independent chains per batch group halve the sequential depth.

The gold (numerator) score is O(B*S) gather work — computed on host.
"""
import numpy as np

B, S, T = 128, 512, 256
NCORES = 8
BPC = B // NCORES          # batch per core = 16
G = 1                      # batch groups per core (chains = 2*G)
GB = BPC // G
W = 128                    # steps per emissions chunk
R = 0                      # renormalization period (0 = off; drift is ~+-6 nats over a chain, far within f32 range)
DELTA = 6.045              # per-step log-space shift ~ log(256) + 0.5
KEEP_MM_WAITS = True       # skip bacc's move_matmul_waits_to_ldweights

_cache = {}


def build_nc(n_steps=S):
    import concourse.bass as bass
    import concourse.tile as tile
    from concourse import bacc, mybir
    from contextlib import ExitStack

    f32 = mybir.dt.float32
    bf16 = mybir.dt.bfloat16
    Exp = mybir.ActivationFunctionType.Exp
    Ln = mybir.ActivationFunctionType.Ln

    assert n_steps >= 4
    Rf = (n_steps - 2) // 2          # forward DVE-rounds (alpha_1..alpha_Rf)
    Rb = n_steps - 2 - Rf            # backward rounds (u_{S-2}..u_{Rf+1})

    nc = bacc.Bacc()
    em = nc.declare_dram_parameter("em", [2, 128, n_steps, BPC], f32, isOutput=False)
    tr = nc.declare_dram_parameter("tr", [2, 128, 2, 128], f32, isOutput=False)
    trt = nc.declare_dram_parameter("trt", [2, 128, 2, 128], f32, isOutput=False)
    stw = nc.declare_dram_parameter("stw", [2, 128, 1], f32, isOutput=False)
    enw = nc.declare_dram_parameter("enw", [2, 128, 1], f32, isOutput=False)
    out = nc.declare_dram_parameter("out", [1, BPC], f32, isOutput=True)

    with ExitStack() as ctx:
        tc = ctx.enter_context(tile.TileContext(nc))
        const = ctx.enter_context(tc.tile_pool(name="const", bufs=1))
        emf = ctx.enter_context(tc.tile_pool(name="emf", bufs=3))
        eef = ctx.enter_context(tc.tile_pool(name="eef", bufs=3))
        emb = ctx.enter_context(tc.tile_pool(name="emb", bufs=3))
        eeb = ctx.enter_context(tc.tile_pool(name="eeb", bufs=3))
        ppool = ctx.enter_context(tc.tile_pool(name="p", bufs=3))
        rpool = ctx.enter_context(tc.tile_pool(name="rn", bufs=2))
        qpool = ctx.enter_context(tc.tile_pool(name="q", bufs=1, space="PSUM"))
        spool = ctx.enter_context(tc.tile_pool(name="s", bufs=2, space="PSUM"))

        # ---- one-time constants ----
        E = [[None, None], [None, None]]   # E[i][j]: lhsT for forward
        Et = [[None, None], [None, None]]  # Et[i][j]: lhsT for backward
        for i in range(2):
            stage = rpool.tile([128, 2, 128], f32, tag="trstage", name="trstage")
            nc.sync.dma_start(out=stage, in_=tr[i])
            for j in range(2):
                E[i][j] = const.tile([128, 128], bf16, tag=f"E{i}{j}", name=f"E{i}{j}")
                nc.scalar.activation(E[i][j], stage[:, j, :], Exp)
        for i in range(2):
            stage = rpool.tile([128, 2, 128], f32, tag="trstage", name="trstaget")
            nc.sync.dma_start(out=stage, in_=trt[i])
            for j in range(2):
                Et[i][j] = const.tile([128, 128], bf16, tag=f"Et{i}{j}",
                                      name=f"Et{i}{j}")
                nc.scalar.activation(Et[i][j], stage[:, j, :], Exp)
        ones = const.tile([128, 128], bf16, tag="ones", name="ones")
        nc.vector.memset(ones, 1.0)
        onesf = const.tile([128, 1], f32, tag="onesf", name="onesf")
        nc.vector.memset(onesf, 1.0)
        dbias = const.tile([128, 1], f32, tag="dbias", name="dbias")
        nc.vector.memset(dbias, -DELTA)
        st_t = []
        for i in range(2):
            t = const.tile([128, 1], f32, tag=f"st{i}", name=f"st{i}")
            nc.sync.dma_start(out=t, in_=stw[i])
            st_t.append(t)
        ben = []
        for i in range(2):
            stage = rpool.tile([128, 1], f32, tag="enstage", name="enstage")
            nc.sync.dma_start(out=stage, in_=enw[i])
            t = const.tile([128, 1], f32, tag=f"ben{i}", name=f"ben{i}")
            nc.vector.tensor_add(t, stage, dbias)   # end - delta (bwd init bias)
            ben.append(t)
        accs = {}
        for d in ("f", "b"):
            for g in range(G):
                a = const.tile([1, GB], f32, tag=f"acc{d}{g}", name=f"acc{d}{g}")
                nc.vector.memset(a, 1.0)
                accs[(d, g)] = a

        # ---- emissions chunk streaming (per direction) ----
        # Stream each chunk in 16-step pieces (DMA pair + exp ACT per piece),
        # ordered by consumption direction, so the first rounds' ee slices are
        # ready within ~1us of kernel start instead of after the full chunk.
        def load_chunk(c, pool, eepool_, nm, descending=False,
                       first_only=False, tiles=None):
            # emissions DMAs issue from the (otherwise idle) GpSimd engine so
            # they don't serialize behind each other on Sync; each 16-step
            # piece gets its own DMA pair + exp ACT so early rounds' ee
            # slices are ready within ~1us.
            s0, s1 = c * W, min(n_steps, (c + 1) * W)
            n = s1 - s0
            if tiles is None:
                t = pool.tile([128, 2, W, BPC], f32, tag="emchunk",
                              name=f"em{nm}")
                te = eepool_.tile([128, 2, W, BPC], bf16, tag="eechunk",
                                  name=f"ee{nm}")
            else:
                t, te = tiles
            pieces = [(a, min(a + 16, n)) for a in range(0, n, 16)]
            if descending:
                pieces = pieces[::-1]
            if first_only:
                pieces = pieces[:1]
            elif tiles is not None:
                pieces = pieces[1:]
            for a, b in pieces:
                for i in range(2):
                    nc.gpsimd.dma_start(out=t[:, i, a:b, :],
                                        in_=em[i, :, s0 + a:s0 + b, :])
                nc.scalar.activation(te[:, :, a:b, :], t[:, :, a:b, :],
                                     Exp, bias=dbias)
            return t, te

        # ---- chain state ----
        # First the two init-critical pieces + the state inits, then the bulk
        # of both chunks — keeps the first matmul off the DMA/ACT queues.
        cf = 0                       # forward chunk index
        cb = (n_steps - 1) // W      # backward chunk index
        tf = load_chunk(cf, emf, eef, "f0", first_only=True)
        same = (cb == cf)
        tb = tf if same else load_chunk(cb, emb, eeb, "b0", descending=True,
                                        first_only=True)
        em_f, ee_f = tf
        em_b, ee_b = tb

        p = []   # forward states per group
        u = []   # backward states per group
        for g in range(G):
            pt = ppool.tile([128, 2, GB], bf16, tag=f"pf{g}", name=f"pf{g}")
            ut = ppool.tile([128, 2, GB], bf16, tag=f"pb{g}", name=f"pb{g}")
            for i in range(2):
                nc.scalar.activation(pt[:, i, :],
                                     em_f[:, i, 0, g * GB:(g + 1) * GB],
                                     Exp, bias=st_t[i])
                nc.scalar.activation(ut[:, i, :],
                                     em_b[:, i, (n_steps - 1) % W,
                                          g * GB:(g + 1) * GB],
                                     Exp, bias=ben[i])
            p.append(pt)
            u.append(ut)
        load_chunk(cf, emf, eef, "f0", tiles=tf)
        if not same:
            load_chunk(cb, emb, eeb, "b0", descending=True, tiles=tb)

        recf = [None] * G
        recb = [None] * G

        def chain_round(g, state, Emat, qtag, ee_t, w, rec, nm):
            """One MM+DVE round for one chain; returns (new_state, q tiles)."""
            q0 = qpool.tile([128, GB], f32, tag=f"{qtag}0", name=f"{qtag}0")
            q1 = qpool.tile([128, GB], f32, tag=f"{qtag}1", name=f"{qtag}1")
            for j, qj in enumerate((q0, q1)):
                for i in range(2):
                    nc.tensor.matmul(qj, Emat[i][j], state[:, i, :],
                                     start=(i == 0), stop=(i == 1))
            newt = ppool.tile([128, 2, GB], bf16, tag=nm, name=nm)
            for j, qj in enumerate((q0, q1)):
                eesl = ee_t[:, j, w, g * GB:(g + 1) * GB]
                if rec is not None:
                    ee2 = rpool.tile([128, GB], bf16, tag=f"sc{nm}{j}",
                                     name=f"sc{nm}{j}")
                    nc.vector.tensor_mul(ee2, eesl, rec)
                    eesl = ee2
                nc.vector.tensor_mul(newt[:, j, :], qj, eesl)
            return newt

        def renorm(g, state, d):
            sp = spool.tile([128, GB], f32, tag="rsum", name=f"rsum{d}{g}")
            for i in range(2):
                nc.tensor.matmul(sp, ones, state[:, i, :],
                                 start=(i == 0), stop=(i == 1))
            rc = rpool.tile([128, GB], f32, tag=f"rc{d}{g}", name=f"rc{d}{g}")
            nc.vector.reciprocal(rc, sp)
            nc.vector.tensor_mul(accs[(d, g)], accs[(d, g)], sp[0:1, :])
            return rc

        # chunk bookkeeping: prefetch the next chunk half-way through the
        # current one (pools are triple-buffered), switch refs at boundaries
        fwd_tiles = {cf: (em_f, ee_f)}
        bwd_tiles = {cb: (em_b, ee_b)}
        cf_hi, cb_lo = cf, cb
        n_rounds = max(Rf, Rb)
        for r in range(1, n_rounds + 1):
            sf = r                     # forward step index (uses ee_sf)
            sb = n_steps - 1 - r       # backward: produces u_sb using ee_sb
            if sf <= Rf:
                ahead = min((sf + W // 2) // W, Rf // W)
                if ahead > cf_hi:
                    cf_hi = ahead
                    fwd_tiles[ahead] = load_chunk(ahead, emf, eef, f"f{ahead}")
                em_f, ee_f = fwd_tiles[sf // W]
            if sb >= Rf + 1:
                behind = max((sb - W // 2) // W, (Rf + 1) // W)
                if behind < cb_lo:
                    cb_lo = behind
                    bwd_tiles[behind] = load_chunk(behind, emb, eeb,
                                                   f"b{behind}",
                                                   descending=True)
                em_b, ee_b = bwd_tiles[sb // W]
            for g in range(G):
                if sf <= Rf:
                    p[g] = chain_round(g, p[g], E, f"qf{g}", ee_f, sf % W,
                                       recf[g], f"pf{g}")
                    recf[g] = None
                if sb >= Rf + 1:
                    u[g] = chain_round(g, u[g], Et, f"qb{g}", ee_b, sb % W,
                                       recb[g], f"pb{g}")
                    recb[g] = None
            if R and r % R == 0:
                for g in range(G):
                    if sf < Rf:
                        recf[g] = renorm(g, p[g], "f")
                    if sb > Rf + 1:
                        recb[g] = renorm(g, u[g], "b")

        # ---- final: Z = (E^T alpha_Rf)^T u_{Rf+1} ----
        for g in range(G):
            q0 = qpool.tile([128, GB], f32, tag=f"qf{g}0", name=f"qfin{g}0")
            q1 = qpool.tile([128, GB], f32, tag=f"qf{g}1", name=f"qfin{g}1")
            for j, qj in enumerate((q0, q1)):
                for i in range(2):
                    nc.tensor.matmul(qj, E[i][j], p[g][:, i, :],
                                     start=(i == 0), stop=(i == 1))
            d = rpool.tile([128, 2, GB], f32, tag=f"d{g}", name=f"d{g}")
            nc.vector.tensor_mul(d[:, 0, :], q0, u[g][:, 0, :])
            nc.vector.tensor_mul(d[:, 1, :], q1, u[g][:, 1, :])
            fin = spool.tile([1, GB], f32, tag="fin", name=f"fin{g}")
            for i in range(2):
                nc.tensor.matmul(fin, onesf, d[:, i, :],
                                 start=(i == 0), stop=(i == 1))
            res = rpool.tile([1, GB], f32, tag=f"res{g}", name=f"res{g}")
            nc.vector.tensor_mul(res, fin, accs[("f", g)])
            res2 = rpool.tile([1, GB], f32, tag=f"res2{g}", name=f"res2{g}")
            nc.vector.tensor_mul(res2, res, accs[("b", g)])
            nc.sync.dma_start(out=out[0:1, g * GB:(g + 1) * GB], in_=res2)

    if KEEP_MM_WAITS:
        nc.move_matmul_waits_to_ldweights = lambda: None
    nc.compile()
    return nc


def _prep_inputs(emissions, transitions, start_transitions, end_transitions,
                 n_steps=S):
    """Host-side layout prep: per-core input maps."""
    emissions = np.ascontiguousarray(emissions[:, :n_steps, :], dtype=np.float32)
    em_t = np.ascontiguousarray(emissions.transpose(2, 1, 0)).reshape(
        2, 128, n_steps, B)  # [i, p, s, b]
    trm = np.asarray(transitions, np.float32)
    tr = np.ascontiguousarray(trm.reshape(2, 128, 2, 128))
    trt = np.ascontiguousarray(trm.T.reshape(2, 128, 2, 128))
    stw = np.ascontiguousarray(
        np.asarray(start_transitions, np.float32).reshape(2, 128, 1))
    enw = np.ascontiguousarray(
        np.asarray(end_transitions, np.float32).reshape(2, 128, 1))
    in_maps = []
    for c in range(NCORES):
        in_maps.append({
            "em": np.ascontiguousarray(em_t[:, :, :, c * BPC:(c + 1) * BPC]),
            "tr": tr, "trt": trt, "stw": stw, "enw": enw,
        })
    return in_maps


def _gold_score_host(emissions, tags, mask, transitions, start_transitions,
                     end_transitions):
    emissions = np.asarray(emissions, np.float32)
    tags = np.asarray(tags, np.int64)
    m = np.asarray(mask, np.float32)
    emit = np.take_along_axis(emissions, tags[..., None], axis=2)[..., 0]
    trans = np.asarray(transitions, np.float32)[tags[:, :-1], tags[:, 1:]]
    score = (np.asarray(start_transitions, np.float32)[tags[:, 0]] + emit[:, 0]
             + ((emit[:, 1:] + trans) * m[:, 1:]).sum(axis=1))
    last_idx = np.asarray(mask, np.int64).sum(axis=1) - 1
    last_tags = np.take_along_axis(tags, last_idx[:, None], axis=1)[:, 0]
    return score + np.asarray(end_transitions, np.float32)[last_tags]


def _numpy_fallback(emissions, tags, mask, transitions, start_transitions,
                    end_transitions):
    """Reference-faithful numpy path (only used if mask is not all ones)."""
    em = np.asarray(emissions, np.float64)
    msk = np.asarray(mask, bool)
    trn = np.asarray(transitions, np.float64)
    alpha = np.asarray(start_transitions, np.float64)[None, :] + em[:, 0]
    for s in range(1, em.shape[1]):
        scores = alpha[:, :, None] + trn[None, :, :] + em[:, s][:, None, :]
        mx = scores.max(axis=1, keepdims=True)
        new = np.log(np.exp(scores - mx).sum(axis=1)) + mx[:, 0, :]
        alpha = np.where(msk[:, s][:, None], new, alpha)
    fin = alpha + np.asarray(end_transitions, np.float64)[None, :]
    mx = fin.max(axis=1, keepdims=True)
    logden = np.log(np.exp(fin - mx).sum(axis=1)) + mx[:, 0]
    gold = _gold_score_host(emissions, tags, mask, transitions,
                            start_transitions, end_transitions)
    return np.array(np.mean(gold - logden), dtype=np.float32)


def run_device(emissions, transitions, start_transitions, end_transitions,
               n_steps=S, trace=False, tmpdir=None):
    """Compile (cached) + run the Bass kernel; returns (logden[B], results_obj)."""
    from concourse.bass_utils import run_bass_kernel_spmd
    key = n_steps
    if key not in _cache:
        _cache[key] = build_nc(n_steps)
    nc = _cache[key]
    in_maps = _prep_inputs(emissions, transitions, start_transitions,
                           end_transitions, n_steps)
    core_ids = list(range(NCORES))
    r = run_bass_kernel_spmd(nc, in_maps, core_ids, trace=trace, tmpdir=tmpdir)
    zprod = np.concatenate([np.asarray(r.results[c]["out"][0], np.float32)
                            for c in range(NCORES)])
    logden = np.log(zprod) + np.float32((n_steps - 1) * DELTA)
    return logden, r


def kernel(emissions, tags, mask, transitions, start_transitions,
           end_transitions):
    emissions = np.asarray(emissions)
    tags = np.asarray(tags)
    mask = np.asarray(mask)
    if not mask.all():
        return _numpy_fallback(emissions, tags, mask, transitions,
                               start_transitions, end_transitions)
    logden, _ = run_device(emissions, transitions, start_transitions,
                           end_transitions)
    gold = _gold_score_host(emissions, tags, mask, transitions,
                            start_transitions, end_transitions)
    return np.array(np.mean(gold - logden), dtype=np.float32)
